# revision 25
# baseline (speedup 1.0000x reference)
"""Tensor-parallel LlamaAttention (S=2048, HID=4096, NH=32, NKV=8) on 8 trn2 cores.

Sharding: core c owns q heads {c, c+8, c+16, c+24} (head h uses kv head h%8,
so all four share kv head c) and kv head c.  Projections + attention are fully
local; avT (bf16, [128d, 2048s] per head) is AllGathered per head-group, then
each core computes its 512 output columns of o_proj (column-parallel wo).

Self-contained: shapes/sharding hardcoded; host does transposes/casts.
"""

import numpy as np
import ml_dtypes

import concourse.bacc as bacc
import concourse.tile as tile
import concourse.mybir as mybir
from concourse.bass_utils import run_bass_kernel_spmd

S = 2048
HID = 4096
NH = 32
NKV = 8
HD = 128
HALF = 64
N_CORES = 8
NREP = NH // NKV  # 4 q heads per core
NHT = HID // 128  # 32 hidden tiles
NST = S // 128    # 16 seq tiles
NSC = S // 512    # 4 seq chunks
BF16 = mybir.dt.bfloat16
F32 = mybir.dt.float32

_CACHE = {}


def build_nc():
    nc = bacc.Bacc("TRN2", target_bir_lowering=False, debug=False,
                   num_devices=N_CORES)

    xT = nc.dram_tensor("xT", [HID, S], BF16, kind="ExternalInput").ap()
    wq = nc.dram_tensor("wqT", [HID, NREP * HD], BF16, kind="ExternalInput").ap()
    wk = nc.dram_tensor("wkT", [HID, HD], BF16, kind="ExternalInput").ap()
    wv = nc.dram_tensor("wvT", [HID, HD], BF16, kind="ExternalInput").ap()
    wo = nc.dram_tensor("woT", [HID, 512], BF16, kind="ExternalInput").ap()
    cosT = nc.dram_tensor("cosT", [HD, S], BF16, kind="ExternalInput").ap()
    sinT = nc.dram_tensor("sinT", [HD, S], BF16, kind="ExternalInput").ap()
    idT = nc.dram_tensor("idT", [128, 128], BF16, kind="ExternalInput").ap()
    mneg = nc.dram_tensor("mnegT", [128, 128], BF16, kind="ExternalInput").ap()
    ones_c = nc.dram_tensor("ones_c", [128, 1], BF16, kind="ExternalInput").ap()
    ones_r = nc.dram_tensor("ones_r", [1, 128], F32, kind="ExternalInput").ap()

    o_out = nc.dram_tensor("o_out", [S, 512], F32, kind="ExternalOutput").ap()

    ag_in = [nc.dram_tensor(f"ag_in{j}", [HD, S], BF16).ap() for j in range(NREP)]
    ag_out = [nc.dram_tensor(f"ag_out{j}", [N_CORES * HD, S], BF16,
                             addr_space="Shared").ap() for j in range(NREP)]

    with tile.TileContext(nc) as tc:
        _body(nc, tc, xT, wq, wk, wv, wo, cosT, sinT, idT, mneg, ones_c,
              ones_r, o_out, ag_in, ag_out)
    nc.compile()
    return nc


def _body(nc, tc, xT, wq, wk, wv, wo, cosT, sinT, idT, mneg, ones_c,
          ones_r, o_out, ag_in, ag_out):
    with (
        tc.tile_pool(name="consts", bufs=1) as cpool,
        tc.tile_pool(name="wo", bufs=1) as wopool,
    ):
        # ---- small constants (live through phase 2) ----
        id_sb = cpool.tile([128, 128], BF16, tag="id")
        mneg_sb = cpool.tile([128, 128], BF16, tag="mneg")
        onc_sb = cpool.tile([128, 1], BF16, tag="onc")
        onr_sb = cpool.tile([1, 128], F32, tag="onr")
        warm_sb = cpool.tile([128, 128], BF16, tag="warm")
        nc.sync.dma_start(out=id_sb[:], in_=idT[:])
        nc.sync.dma_start(out=mneg_sb[:], in_=mneg[:])
        nc.sync.dma_start(out=onc_sb[:], in_=ones_c[:])
        nc.sync.dma_start(out=onr_sb[:], in_=ones_r[:])

        # PE warm-up: keep the tensor engine busy during the initial input
        # DMA so the HAM clock-gate un-throttles before real work arrives.
        nc.vector.memset(warm_sb[:], 0.0)
        with tc.tile_pool(name="wps", bufs=1, space="PSUM") as wpsum:
            wps = wpsum.tile([128, 512], F32, tag="warm", name="warmps")
            for w in range(96):
                nc.tensor.matmul(wps[:, (w % 4) * 128:(w % 4 + 1) * 128],
                                 warm_sb[:], warm_sb[:], start=True, stop=True)

        wo_sb = wopool.tile([128, NHT * 512], BF16, tag="wo")

        _phases123(nc, tc, xT, wq, wk, wv, wo, cosT, sinT, id_sb, mneg_sb,
                   onc_sb, onr_sb, warm_sb, wo_sb, o_out, ag_in, ag_out)


def _phases123(nc, tc, xT, wq, wk, wv, wo, cosT, sinT, id_sb, mneg_sb,
               onc_sb, onr_sb, warm_sb, wo_sb, o_out, ag_in, ag_out):
    with tc.tile_pool(name="qkv", bufs=1) as qkvpool:
        # ---- projection outputs (resident, bf16) ----
        qT_sb = [qkvpool.tile([HD, S], BF16, tag=f"qT{j}", name=f"qT{j}")
                 for j in range(NREP)]
        kT_sb = qkvpool.tile([HD, S], BF16, tag="kT")
        v_sb = qkvpool.tile([128, S], BF16, tag="v")  # col block kt = s tile kt

        with (
            tc.tile_pool(name="rconsts", bufs=1) as rcpool,
            tc.tile_pool(name="wproj", bufs=1) as wpool,
            tc.tile_pool(name="xc", bufs=64) as xpool,
            tc.tile_pool(name="rope", bufs=2) as rpool,
            tc.tile_pool(name="ps1", bufs=1, space="PSUM") as ps1,
        ):
            _phase1(nc, tc, xT, wq, wk, wv, cosT, sinT, qT_sb, kT_sb, v_sb,
                    rcpool, wpool, xpool, rpool, ps1, warm_sb)

        with tc.tile_pool(name="ag", bufs=1) as agpool:
            agt = []
            for j in range(NREP):
                for r in range(N_CORES):
                    agt.append(agpool.tile([128, S], BF16, tag=f"ag{j}_{r}",
                                           name=f"ag{j}_{r}"))
            # o_proj weights stream in during phase-1 tail / phase 2
            for i in range(NHT):
                nc.sync.dma_start(out=wo_sb[:, i * 512:(i + 1) * 512],
                                  in_=wo[i * 128:(i + 1) * 128, :])
            with (
                tc.tile_pool(name="sc2", bufs=3, space="PSUM") as sc2,
                tc.tile_pool(name="psav", bufs=1, space="PSUM") as psav,
                tc.tile_pool(name="pspb", bufs=1, space="PSUM") as pspb,
                tc.tile_pool(name="probs", bufs=5) as ppool,
                tc.tile_pool(name="avt", bufs=2) as avpool,
                tc.tile_pool(name="small", bufs=2) as spool,
            ):
                _phase2(nc, tc, qT_sb, kT_sb, v_sb, id_sb, mneg_sb, onc_sb,
                        onr_sb, ag_in, ag_out, agt, ppool, avpool, spool,
                        sc2, psav, pspb)
            with (
                tc.tile_pool(name="p3a", bufs=4, space="PSUM") as p3a,
                tc.tile_pool(name="p3b", bufs=4, space="PSUM") as p3b,
                tc.tile_pool(name="oout", bufs=2) as opool,
                tc.tile_pool(name="oacc", bufs=1) as oaccpool,
            ):
                _phase3(nc, tc, wo_sb, o_out, agt, p3a, p3b, opool, oaccpool)


def _phase1(nc, tc, xT, wq, wk, wv, cosT, sinT, qT_sb, kT_sb, v_sb,
            rcpool, wpool, xpool, rpool, ps1, warm_sb):
    # ---- weights (resident; column block h = hidden tile h) ----
    wq_sb = wpool.tile([128, NHT * 512], BF16, tag="wq")
    wk_sb = wpool.tile([128, NHT * 128], BF16, tag="wk")
    wv_sb = wpool.tile([128, NHT * 128], BF16, tag="wv")
    cos_sb = rcpool.tile([HD, S], BF16, tag="cos")
    sin_sb = rcpool.tile([HD, S], BF16, tag="sin")
    wps1 = ps1.tile([128, 512], F32, tag="warm", name="warmps1")

    # DMA issue order is chosen so chunk-0 k/v projections can start early:
    # wk+wv+x(chunk0) interleaved, then rope tables, then wq, then x(1..3).
    xcs0 = [xpool.tile([128, 512], BF16, tag="xc", name=f"xc0_{h}")
            for h in range(NHT)]
    for h in range(NHT):
        nc.sync.dma_start(out=wk_sb[:, h * 128:(h + 1) * 128],
                          in_=wk[h * 128:(h + 1) * 128, :])
        nc.sync.dma_start(out=wv_sb[:, h * 128:(h + 1) * 128],
                          in_=wv[h * 128:(h + 1) * 128, :])
        nc.sync.dma_start(out=xcs0[h][:], in_=xT[h * 128:(h + 1) * 128, 0:512])
    nc.sync.dma_start(out=cos_sb[:], in_=cosT[:])
    nc.sync.dma_start(out=sin_sb[:], in_=sinT[:])
    for h in range(NHT):
        nc.sync.dma_start(out=wq_sb[:, h * 512:(h + 1) * 512],
                          in_=wq[h * 128:(h + 1) * 128, :])

    for cs in range(NSC):
        sc = slice(cs * 512, (cs + 1) * 512)
        if cs == 0:
            xcs = xcs0
        else:
            xcs = [xpool.tile([128, 512], BF16, tag="xc", name=f"xc{cs}_{h}")
                   for h in range(NHT)]
            for h in range(NHT):
                nc.sync.dma_start(out=xcs[h][:],
                                  in_=xT[h * 128:(h + 1) * 128, sc])

        def _rope(dst, pp):
            t1 = rpool.tile([HALF, 512], F32, tag="t1")
            t2 = rpool.tile([HALF, 512], F32, tag="t2")
            nc.vector.tensor_mul(t1[:], pp[0:HALF, :], cos_sb[0:HALF, sc])
            nc.vector.tensor_mul(t2[:], pp[HALF:128, :], sin_sb[0:HALF, sc])
            nc.vector.tensor_sub(dst[0:HALF, sc], t1[:], t2[:])
            t3 = rpool.tile([HALF, 512], F32, tag="t1")
            t4 = rpool.tile([HALF, 512], F32, tag="t2")
            nc.vector.tensor_mul(t3[:], pp[HALF:128, :], cos_sb[HALF:128, sc])
            nc.vector.tensor_mul(t4[:], pp[0:HALF, :], sin_sb[HALF:128, sc])
            nc.vector.tensor_add(dst[HALF:128, sc], t3[:], t4[:])

        # k first (only needs wk + x), then v, then the four q heads.
        # In chunk 0 the PE is paced by the incoming DMA stream, so filler
        # matmuls are interleaved to keep HAM activity high (full clock).
        pk = ps1.tile([128, 512], F32, tag="mm", bufs=4)
        for h in range(NHT):
            nc.tensor.matmul(pk[:], wk_sb[:, h * 128:(h + 1) * 128],
                             xcs[h][:],
                             start=(h == 0), stop=(h == NHT - 1))
            nfill = 10 if cs == 0 else (3 if cs == 1 else 0)
            for _ in range(nfill):
                nc.tensor.matmul(wps1[:, 0:128], warm_sb[:], warm_sb[:],
                                 start=True, stop=True)
        _rope(kT_sb, pk)

        pv = ps1.tile([128, 512], F32, tag="mm", bufs=4)
        for tl in range(4):
            for h in range(NHT):
                nc.tensor.matmul(
                    pv[:, tl * 128:(tl + 1) * 128],
                    xcs[h][:, tl * 128:(tl + 1) * 128],
                    wv_sb[:, h * 128:(h + 1) * 128],
                    start=(h == 0), stop=(h == NHT - 1))
                if cs == 0 and tl < 2:
                    nc.tensor.matmul(wps1[:, 0:128], warm_sb[:], warm_sb[:],
                                     start=True, stop=True)
        nc.scalar.copy(v_sb[:, sc], pv[:])

        for j in range(NREP):
            pq = ps1.tile([128, 512], F32, tag="mm", bufs=4)
            for h in range(NHT):
                nc.tensor.matmul(
                    pq[:],
                    wq_sb[:, h * 512 + j * 128: h * 512 + (j + 1) * 128],
                    xcs[h][:],
                    start=(h == 0), stop=(h == NHT - 1))
                if cs <= 1 and j == 0:
                    for _ in range(2):
                        nc.tensor.matmul(wps1[:, 0:128], warm_sb[:],
                                         warm_sb[:], start=True, stop=True)
            _rope(qT_sb[j], pq)


def _phase2(nc, tc, qT_sb, kT_sb, v_sb, id_sb, mneg_sb, onc_sb, onr_sb,
            ag_in, ag_out, agt, ppool, avpool, spool, sc2, psav, pspb):
    Exp = mybir.ActivationFunctionType.Exp
    # ---- phase 2: attention (scores transposed: [k, sq]) ----
    # kt tiles are processed in pairs sharing one [128,1024] PSUM tile so the
    # exp amortizes ACT overhead.  Causal masking inside diagonal blocks is a
    # -1e9 accumulate matmul (identity stationary), so exp yields exact 0s.
    # Row-sums accumulate on DVE (bf16) with one ones-matmul per chunk.
    # Normalization of chunk (j,C) is deferred into the next chunk's kt loop,
    # split into an early reciprocal and a late broadcast/normalize part.
    DEPTH = 3
    carry_a = [None]
    carry_b = [None]

    def make_rowsum(j, C, R, rrec):
        def f():
            prs_t = pspb.tile([128, 512], F32, tag="pb", name=f"prs{j}_{C}")
            nc.tensor.matmul(prs_t[0:1, :], onc_sb[:], R[:],
                             start=True, stop=True)
            nc.vector.reciprocal_approx_fast(rrec[:], prs_t[0:1, :])
        return f

    def make_norm(j, C, pav_sb, rrec):
        def f():
            qc = slice(C * 512, (C + 1) * 512)
            pb = pspb.tile([128, 512], F32, tag="pb", name=f"pb{j}_{C}")
            nc.tensor.matmul(pb[:], onr_sb[:], rrec[:], start=True, stop=True)
            bsb = spool.tile([128, 512], F32, tag="bsb", name=f"bsb{j}_{C}",
                             bufs=1)
            nc.scalar.copy(bsb[:], pb[:])
            avc = avpool.tile([128, 512], BF16, tag="avc", name=f"avc{j}_{C}")
            nc.vector.tensor_mul(avc[:], pav_sb[:], bsb[:])
            nc.sync.dma_start(out=ag_in[j][:, qc], in_=avc[:])
            if C == NSC - 1:
                nc.gpsimd.collective_compute(
                    "AllGather", mybir.AluOpType.bypass,
                    replica_groups=[list(range(N_CORES))],
                    ins=[ag_in[j][:]], outs=[ag_out[j][:]])
                # stage the PREVIOUS head's gathered avT into SBUF.  Issued
                # on the GpSimd queue AFTER this head's AllGather trigger:
                # the load triggers block until AllGather j-1 lands, and must
                # delay neither the Sync-queue DMAs nor the next collective
                # trigger (each AG then fires as soon as its input is ready).
                if j > 0:
                    jp = j - 1
                    for r in range(N_CORES):
                        nc.gpsimd.dma_start(
                            out=agt[jp * N_CORES + r][:],
                            in_=ag_out[jp][r * 128:(r + 1) * 128, :])
        return f

    for j in range(NREP):
        for C in range(NSC):
            qc0 = C * 512
            nkt = 4 * C + 4
            pav = psav.tile([128, 512], F32, tag="av", name=f"pav{j}_{C}",
                            bufs=1)
            R = spool.tile([128, 512], BF16, tag="rsum", name=f"rs{j}_{C}")
            pend = []

            def drain_one():
                kt2, pt2, hp2 = pend.pop(0)
                nc.tensor.matmul(pav[:], v_sb[:, kt2 * 128:(kt2 + 1) * 128],
                                 pt2[:, hp2 * 512:(hp2 + 1) * 512],
                                 start=(kt2 == 0), stop=(kt2 == nkt - 1))

            for p in range(nkt // 2):
                kts = (2 * p, 2 * p + 1)
                offs = [max(0, (kt - 4 * C) * 128) for kt in kts]
                diag = [kt >= 4 * C for kt in kts]
                ps2 = sc2.tile([128, 1024], F32, tag="sc", name=f"s{j}_{C}_{p}")
                for hp, kt in enumerate(kts):
                    off = offs[hp]
                    base = hp * 512
                    nc.tensor.matmul(ps2[:, base + off:base + 512],
                                     kT_sb[:, kt * 128:(kt + 1) * 128],
                                     qT_sb[j][:, qc0 + off: qc0 + 512],
                                     start=True, stop=not diag[hp])
                    if diag[hp]:
                        nc.tensor.matmul(ps2[:, base + off:base + off + 128],
                                         id_sb[:], mneg_sb[:],
                                         start=False, stop=True)
                pt2 = ppool.tile([128, 1024], BF16, tag="pt",
                                 name=f"pt{j}_{C}_{p}")
                if offs[1] == 0:
                    nc.scalar.activation(pt2[:, 0:1024], ps2[:, 0:1024], Exp)
                else:
                    nc.scalar.activation(pt2[:, offs[0]:512],
                                         ps2[:, offs[0]:512], Exp)
                    nc.scalar.activation(pt2[:, 512 + offs[1]:1024],
                                         ps2[:, 512 + offs[1]:1024], Exp)
                    if offs[0] > 0:
                        nc.vector.memset(pt2[:, 0:offs[0]], 0.0)
                    nc.vector.memset(pt2[:, 512:512 + offs[1]], 0.0)
                if kts[0] == 0:
                    nc.vector.tensor_copy(R[:], pt2[:, 0:512])
                else:
                    nc.vector.tensor_add(R[:], R[:], pt2[:, 0:512])
                nc.vector.tensor_add(R[:], R[:], pt2[:, 512:1024])
                pend.append((kts[0], pt2, 0))
                pend.append((kts[1], pt2, 1))
                if p == 0 and carry_a[0] is not None:
                    carry_a[0]()
                    carry_a[0] = None
                if p == 1 and carry_b[0] is not None:
                    carry_b[0]()
                    carry_b[0] = None
                while len(pend) > DEPTH:
                    drain_one()
            while pend:
                drain_one()
            # evacuate the attention accumulator so its PSUM bank frees for
            # the next chunk (psav runs with a single bank)
            pav_sb = spool.tile([128, 512], BF16, tag="pavs",
                                name=f"pavs{j}_{C}")
            nc.vector.tensor_copy(pav_sb[:], pav[:])
            rrec = spool.tile([1, 512], F32, tag="rrec", name=f"rrec{j}_{C}")
            carry_a[0] = make_rowsum(j, C, R, rrec)
            carry_b[0] = make_norm(j, C, pav_sb, rrec)
    carry_a[0]()
    carry_a[0] = None
    carry_b[0]()
    carry_b[0] = None
    # stage the last head's gathered avT in column-chunks so o_proj can
    # start before the full tensor arrives; alternate DMA queues for
    # throughput (sync is idle by now and o_out writes only start later).
    j = NREP - 1
    for cc in range(4):
        ccs = slice(cc * 512, (cc + 1) * 512)
        for r in range(N_CORES):
            eng = nc.sync if (r % 2 == 0) else nc.gpsimd
            eng.dma_start(out=agt[j * N_CORES + r][:, ccs],
                          in_=ag_out[j][r * 128:(r + 1) * 128, ccs])


def _phase3(nc, tc, wo_sb, o_out, agt, p3a, p3b, opool, oaccpool):
    # ---- phase 3: column-parallel o_proj, two passes ----
    # Pass 1 accumulates head-groups 0-2 for ALL seq tiles (this work is
    # available while the last AllGather is still in flight) and parks the
    # partials in SBUF; pass 2 adds head-group 3 once its gather lands.
    o_acc = [oaccpool.tile([128, 512], BF16, tag=f"oacc{st}", name=f"oacc{st}")
             for st in range(NST)]
    # One sub-pass per head group, so no seq tile ever blocks waiting for a
    # late AllGather while holding a PSUM bank (that stall un-warms HAM and
    # the tail then runs at half clock).  Each group's gather lands while
    # the previous sub-pass computes.
    for g in range(NREP):
        ilo, ihi = g * N_CORES, (g + 1) * N_CORES
        for st in range(NST):
            pool = p3a if (st % 2 == 0) else p3b
            po = pool.tile([128, 512], F32, tag="mm3", name=f"po{g}_{st}")
            for i in range(ilo, ihi):
                nc.tensor.matmul(po[:], agt[i][:, st * 128:(st + 1) * 128],
                                 wo_sb[:, i * 512:(i + 1) * 512],
                                 start=(i == ilo), stop=(i == ihi - 1))
            if g == 0:
                nc.vector.tensor_copy(o_acc[st][:], po[:])
            elif g < NREP - 1:
                nc.vector.tensor_add(o_acc[st][:], o_acc[st][:], po[:])
            else:
                osb = opool.tile([128, 512], F32, tag="o")
                nc.vector.tensor_add(osb[:], po[:], o_acc[st][:])
                nc.sync.dma_start(out=o_out[st * 128:(st + 1) * 128, :],
                                  in_=osb[:])


def prep_inputs(hidden_states, wq, wk, wv, wo, cos, sin, causal_mask=None):
    bf16 = ml_dtypes.bfloat16
    x = np.asarray(hidden_states, np.float32)[0]          # (S, HID)
    xT = np.ascontiguousarray(x.T).astype(bf16)           # (HID, S)
    wq_s = (np.asarray(wq, np.float32) / np.sqrt(HD)).astype(np.float32)
    cos2 = np.asarray(cos, np.float32)[0, 0]              # (S, 64)
    sin2 = np.asarray(sin, np.float32)[0, 0]
    cosT = np.ascontiguousarray(
        np.concatenate([cos2.T, cos2.T], 0)).astype(bf16)  # (128, S)
    sinT = np.ascontiguousarray(
        np.concatenate([sin2.T, sin2.T], 0)).astype(bf16)
    kl = np.arange(128)[:, None]
    ql = np.arange(128)[None, :]
    idT = np.eye(128, dtype=bf16)                         # identity stationary
    mnegT = np.where(kl > ql, -1e9, 0.0).astype(bf16)     # mask k > q
    ones_c = np.ones((128, 1), bf16)
    ones_r = np.ones((1, 128), np.float32)

    # wo reordered to match AllGather row order: row p = j*1024 + r*128 + d
    # corresponds to head (j*8+r), dim d  ->  wo column (j*8+r)*128 + d.
    j_ = np.arange(NREP)[:, None, None]
    r_ = np.arange(N_CORES)[None, :, None]
    d_ = np.arange(HD)[None, None, :]
    col_order = ((j_ * N_CORES + r_) * HD + d_).reshape(-1)
    woT_full = np.ascontiguousarray(
        np.asarray(wo, np.float32)[:, col_order].T).astype(bf16)  # (4096c, 4096hid)

    in_maps = []
    for c in range(N_CORES):
        heads = [jj * N_CORES + c for jj in range(NREP)]
        wq_rows = np.concatenate([wq_s[h * HD:(h + 1) * HD, :] for h in heads], 0)
        wqT_c = np.ascontiguousarray(wq_rows.T).astype(bf16)        # (HID, 512)
        wkT_c = np.ascontiguousarray(
            np.asarray(wk, np.float32)[c * HD:(c + 1) * HD, :].T).astype(bf16)
        wvT_c = np.ascontiguousarray(
            np.asarray(wv, np.float32)[c * HD:(c + 1) * HD, :].T).astype(bf16)
        woT_c = np.ascontiguousarray(woT_full[:, c * 512:(c + 1) * 512])
        in_maps.append(dict(xT=xT, wqT=wqT_c, wkT=wkT_c, wvT=wvT_c, woT=woT_c,
                            cosT=cosT, sinT=sinT, idT=idT, mnegT=mnegT,
                            ones_c=ones_c, ones_r=ones_r))
    return in_maps


def postprocess(results):
    out = np.empty((S, HID), np.float32)
    for c in range(N_CORES):
        out[:, c * 512:(c + 1) * 512] = results[c]["o_out"]
    return out[None]


def get_nc():
    if "nc" not in _CACHE:
        _CACHE["nc"] = build_nc()
    return _CACHE["nc"]


def kernel(hidden_states, wq, wk, wv, wo, cos, sin, causal_mask=None):
    nc = get_nc()
    in_maps = prep_inputs(hidden_states, wq, wk, wv, wo, cos, sin, causal_mask)
    res = run_bass_kernel_spmd(nc, in_maps, core_ids=list(range(N_CORES)))
    return postprocess(res.results)
